# revision 1
# baseline (speedup 1.0000x reference)
"""GCN node classifier on 8 Trainium2 NeuronCores.

3-layer GCN, N=100000 nodes, E=3.2M edges, d_in=512, d_h=32, n_cls=40.

Math refactor (linearity of gcn_conv):
    segsum(norm * (h@W)[src]) + b  ==  (dinv ⊙ segsum((dinv ⊙ h)[src]) ) @ W + b
so every per-edge gather moves 32-wide rows and per-edge `norm` disappears
into node-level dinv scaling; the added self-loop becomes a node-local term.

Sharding: nodes (rows) are dst-sharded 12500/core. Each core aggregates all
edges whose dst lies in its shard, gathering source features from a
replicated [N+1, 32] bf16 table rebuilt per layer via AllGather.

Device pipeline per layer:
  gather: gpsimd indirect DMA, h_table[src] -> slot-packed bf16 msg tiles
  scatter: one-hot segment matmul on TensorE accumulating into aligned
           128-node PSUM windows (fixed R rows per window so the SPMD
           program is identical on every core), DVE fold 8->1
  then: + self term, x dinv, @W (+bias), pairnorm stats (matmul colsums +
        tiny AllReduce), relu, rebuild table.
"""

import math
import numpy as np
import ml_dtypes

BF16 = ml_dtypes.bfloat16

# ---------------------------------------------------------------- config

class Cfg:
    def __init__(self, n_nodes, n_edges, d_in=512, d_h=32, n_cls=40, n_cores=8):
        assert n_nodes % n_cores == 0
        self.N = n_nodes
        self.E = n_edges
        self.C = n_cores
        self.NL = n_nodes // n_cores          # nodes per core
        self.TN = math.ceil(self.NL / 128)    # node tiles (=PSUM windows) per core
        self.NLP = self.TN * 128              # padded nodes per core
        self.D_IN = d_in
        self.DH = d_h
        self.DC = n_cls
        self.DUMMY = n_nodes                  # zero row of the table
        self.EPS = 1e-5

FULL = Cfg(100000, 3200000)

# ------------------------------------------------------- host preprocessing

def preprocess(cfg, edge_index):
    """Build per-core gather indices + segment metadata.

    Returns dict with R (rows per 128-node window, multiple of 128) and
    per-core arrays: gidx [128, 8*NSUB] int32, segrel [128, NSUB] bf16,
    dinv_nm [128, TN] f32.
    """
    N, C, NL, TN = cfg.N, cfg.C, cfg.NL, cfg.TN
    src = np.asarray(edge_index[0], dtype=np.int64)
    dst = np.asarray(edge_index[1], dtype=np.int64)
    deg = np.bincount(dst, minlength=N).astype(np.float64) + 1.0  # + self loop
    dinv = (1.0 / np.sqrt(deg)).astype(np.float32)

    core_of = dst // NL
    # per (core, node) edge lists, sorted by (dst, src)
    order = np.lexsort((src, dst))
    s_sorted = src[order]
    d_sorted = dst[order]
    # row counts per node (pad in-degree to multiple of 4)
    cnt = np.bincount(dst, minlength=N)
    rows_per_node = (cnt + 3) // 4
    # rows per (core, window)
    R = 0
    for c in range(C):
        for w in range(TN):
            lo, hi = c * NL + w * 128, min(c * NL + (w + 1) * 128, (c + 1) * NL)
            R = max(R, int(rows_per_node[lo:hi].sum()))
    R = max(256, ((R + 127) // 128) * 128)
    KW = R // 128                 # subtiles per window
    NSUB = TN * KW                # subtiles per core per layer

    # start offset of each node's edges in the sorted arrays
    starts = np.zeros(N + 1, dtype=np.int64)
    np.cumsum(cnt, out=starts[1:])

    gidx = np.full((C, 128, 4 * NSUB), cfg.DUMMY, dtype=np.int32)
    segrel = np.zeros((C, 128, NSUB), dtype=np.float32)
    for c in range(C):
        for w in range(TN):
            cur = 0                       # row cursor within window
            base_row = w * R
            lo = c * NL + w * 128
            hi = min(c * NL + (w + 1) * 128, (c + 1) * NL)
            for v in range(lo, hi):
                k = int(rows_per_node[v])
                if k == 0:
                    continue
                e0, e1 = starts[v], starts[v + 1]
                srcs = s_sorted[e0:e1]
                vrel = v - lo
                for r in range(k):
                    g = base_row + cur
                    S, p = g // 128, g % 128
                    segrel[c, p, S] = vrel
                    chunkv = srcs[4 * r: 4 * r + 4]
                    gidx[c, p, 4 * S: 4 * S + len(chunkv)] = chunkv
                    cur += 1
            assert cur <= R

    dinv_nm = np.zeros((C, 128, TN), dtype=np.float32)
    for c in range(C):
        v = c * NL + np.arange(cfg.NLP)
        valid = v < (c + 1) * NL
        dd = np.where(valid, dinv[np.minimum(v, N - 1)], 0.0)
        dinv_nm[c] = dd.reshape(TN, 128).T
    return dict(R=R, KW=KW, NSUB=NSUB, gidx=gidx, segrel=segrel.astype(BF16),
                dinv_nm=dinv_nm)


# ------------------------------------------------------------ device program

def build_program(cfg, R, debug=False):
    """Build the SPMD bass program. Returns (nc, names) where names lists
    the dram parameter names in input order."""
    from concourse import bass, bacc, mybir, tile
    from concourse.masks import make_identity

    f32 = mybir.dt.float32
    bf16 = mybir.dt.bfloat16
    i32 = mybir.dt.int32

    N, NL, NLP, TN, D_IN, DH, DC = (cfg.N, cfg.NL, cfg.NLP, cfg.TN,
                                    cfg.D_IN, cfg.DH, cfg.DC)
    KW = R // 128
    NSUB = TN * KW
    KC = D_IN // 128              # contraction chunks for x@W0
    CH_W = 4                      # windows per gather chunk
    rg = [list(range(cfg.C))]

    nc = bacc.Bacc("TRN2", target_bir_lowering=False, debug=False,
                   num_devices=cfg.C)

    # -------- kernel I/O
    x_bf = nc.dram_tensor("x_bf", [NLP, D_IN], bf16, kind="ExternalInput").ap()
    gidx_d = nc.dram_tensor("gidx", [128, 4 * NSUB], i32, kind="ExternalInput").ap()
    segrel_d = nc.dram_tensor("segrel", [128, NSUB], bf16, kind="ExternalInput").ap()
    dinv_d = nc.dram_tensor("dinv_nm", [128, TN], f32, kind="ExternalInput").ap()
    w0_d = nc.dram_tensor("w0", [D_IN, DH], bf16, kind="ExternalInput").ap()
    w1_d = nc.dram_tensor("w1", [DH, DH], f32, kind="ExternalInput").ap()
    wf_d = nc.dram_tensor("wf", [DH, DC], f32, kind="ExternalInput").ap()
    b0_d = nc.dram_tensor("b0", [128, DH], f32, kind="ExternalInput").ap()
    b1_d = nc.dram_tensor("b1", [128, DH], f32, kind="ExternalInput").ap()
    bf_d = nc.dram_tensor("bf_", [128, DC], f32, kind="ExternalInput").ap()
    iota_d = nc.dram_tensor("iota_row", [128, 128], bf16, kind="ExternalInput").ap()
    out_d = nc.dram_tensor("out", [NL, DC], f32, kind="ExternalOutput").ap()

    if debug:
        dbg_hloc = nc.dram_tensor("dbg_hloc", [128, TN * DH], f32,
                                  kind="ExternalOutput").ap()
        dbg_msg = nc.dram_tensor("dbg_msg", [128, 4 * (R // 128) * 8 * DH], bf16,
                                 kind="ExternalOutput").ap()
        dbg_agg = nc.dram_tensor("dbg_agg", [128, TN * DH], f32,
                                 kind="ExternalOutput").ap()

    # -------- internal DRAM
    table = nc.dram_tensor("h_table", [N + 1, DH], bf16).ap()
    shard = nc.dram_tensor("h_shard", [NLP, DH], bf16).ap()
    st_in = nc.dram_tensor("st_in", [1, 64], f32).ap()
    st_out = nc.dram_tensor("st_out", [1, 64], f32).ap()

    names = ["x_bf", "gidx", "segrel", "dinv_nm", "w0", "w1", "wf",
             "b0", "b1", "bf_", "iota_row"]

    # gather chunking over windows
    chunks = []
    w0c = 0
    while w0c < TN:
        nw = min(CH_W, TN - w0c)
        chunks.append((w0c, nw))
        w0c += nw

    with tile.TileContext(nc) as tc:
        import contextlib
        with contextlib.ExitStack() as ctx:
            big = ctx.enter_context(tc.tile_pool(name="big", bufs=1))
            msgp = ctx.enter_context(tc.tile_pool(name="msg", bufs=3))
            ohp = ctx.enter_context(tc.tile_pool(name="oh", bufs=3))
            smp = ctx.enter_context(tc.tile_pool(name="sm", bufs=4))
            psw = ctx.enter_context(tc.tile_pool(name="psw", bufs=2, space="PSUM"))
            pstr = ctx.enter_context(tc.tile_pool(name="pstr", bufs=2, space="PSUM"))
            psst = ctx.enter_context(tc.tile_pool(name="psst", bufs=1, space="PSUM"))
            psA = ctx.enter_context(tc.tile_pool(name="psA", bufs=2, space="PSUM"))

            # ---- residents
            gidx_sb = big.tile([128, 4 * NSUB], i32)
            segrel_sb = big.tile([128, NSUB], bf16)
            dinv_sb = big.tile([128, TN], f32)
            w0_sb = big.tile([128, D_IN // 128, DH], bf16)   # [:, kc, :] chunks
            w1_sb = big.tile([DH, DH], f32)
            wf_sb = big.tile([DH, DC], f32)
            b0_sb = big.tile([128, DH], f32)
            b1_sb = big.tile([128, DH], f32)
            bf_sb = big.tile([128, DC], f32)
            iota_sb = big.tile([128, 128], bf16)
            ident = big.tile([128, 128], f32)
            ones_sb = big.tile([128, 1], f32)
            ones_row = big.tile([1, 128], f32)
            zrow = big.tile([1, DH], bf16)
            stats_sb = big.tile([1, 64], f32)

            hloc = big.tile([128, TN, DH], f32)     # dinv * h  (self term + AG src)
            agg = big.tile([128, TN, DH], f32)
            state = big.tile([128, TN, DH], f32)    # out_k then z_k
            shard_sb = big.tile([128, TN, DH], bf16)
            final_sb = big.tile([128, TN, DC], f32)
            sq = final_sb[:, :, 0:DH]               # scratch; real use is layer 2

            nc.sync.dma_start(out=gidx_sb[:], in_=gidx_d[:])
            nc.sync.dma_start(out=segrel_sb[:], in_=segrel_d[:])
            nc.sync.dma_start(out=dinv_sb[:], in_=dinv_d[:])
            nc.sync.dma_start(out=w0_sb[:], in_=w0_d.rearrange("(c p) f -> p c f", p=128))
            nc.sync.dma_start(out=w1_sb[:], in_=w1_d[:])
            nc.sync.dma_start(out=wf_sb[:], in_=wf_d[:])
            nc.sync.dma_start(out=b0_sb[:], in_=b0_d[:])
            nc.sync.dma_start(out=b1_sb[:], in_=b1_d[:])
            nc.sync.dma_start(out=bf_sb[:], in_=bf_d[:])
            nc.sync.dma_start(out=iota_sb[:], in_=iota_d[:])
            make_identity(nc, ident[:])
            nc.vector.memset(ones_sb[:], 1.0)
            nc.vector.memset(ones_row[:], 1.0)
            nc.vector.memset(zrow[:], 0.0)
            nc.sync.dma_start(out=table[cfg.DUMMY:cfg.DUMMY + 1, :], in_=zrow[:])

            def dinv_b(ap_shape):
                return dinv_sb[:, :, None].to_broadcast(ap_shape)

            # ---------------- phase A: t0 = x @ W0 (bf16), hloc = dinv*t0
            TH0 = ((TN + 1) // 2)
            with tc.tile_pool(name="xTp", bufs=1) as xTp:
                for h, (tlo, thi) in enumerate([(0, TH0), (TH0, TN)]):
                    nh = thi - tlo
                    if nh <= 0:
                        continue
                    xT = [xTp.tile([128, TH0 * 128], bf16, tag=f"xT{c}",
                                   name=f"xT{c}_{h}") for c in range(KC)]
                    for c in range(KC):
                        nc.sync.dma_start_transpose(
                            out=xT[c][:, 0:nh * 128],
                            in_=x_bf[tlo * 128: thi * 128,
                                     128 * c:128 * (c + 1)])
                    for t in range(tlo, thi):
                        t0 = psA.tile([128, DH], f32, tag="psA", name=f"t0_{t}")
                        for c in range(KC):
                            nc.tensor.matmul(
                                out=t0[:],
                                lhsT=xT[c][:, 128 * (t - tlo):128 * (t - tlo + 1)],
                                rhs=w0_sb[:, c, :],
                                start=(c == 0), stop=(c == KC - 1))
                        nc.vector.tensor_tensor(
                            out=hloc[:, t, :], in0=t0[:],
                            in1=dinv_sb[:, t:t + 1].to_broadcast([128, DH]),
                            op=mybir.AluOpType.mult)

            def write_table_and_allgather():
                nc.vector.tensor_copy(out=shard_sb[:], in_=hloc[:])
                nc.sync.dma_start(
                    out=shard.rearrange("(t p) f -> p t f", p=128),
                    in_=shard_sb[:])
                nc.gpsimd.collective_compute(
                    "AllGather", mybir.AluOpType.bypass,
                    replica_groups=rg,
                    ins=[shard[0:NL, :]],
                    outs=[table[0:N, :]],
                )

            if debug:
                nc.sync.dma_start(out=dbg_hloc[:],
                                  in_=hloc[:].rearrange("p t f -> p (t f)"))

            write_table_and_allgather()

            # ---------------- layers
            for layer in range(3):
                # ---- gather + segment-sum into agg
                for (wc, nw) in chunks:
                    msg = msgp.tile([128, CH_W * KW, 4, DH], bf16, tag="msg")
                    # HW indirect DMA consumes exactly one index per idx-AP
                    # partition row -> gather 128 rows per instruction.
                    for s in range(nw * KW):
                        for j in range(4):
                            col = 4 * KW * wc + 4 * s + j
                            nc.gpsimd.indirect_dma_start(
                                out=msg[:, s, j, 0:DH],
                                out_offset=None,
                                in_=table[:],
                                in_offset=bass.IndirectOffsetOnAxis(
                                    ap=gidx_sb[:, col:col + 1], axis=0),
                            )
                    if debug and layer == 0 and wc == 0:
                        nc.sync.dma_start(
                            out=dbg_msg[:, 0:nw * KW * 8 * DH].rearrange(
                                "p (s f) -> p s f", f=DH),
                            in_=msg[:, 0:nw * KW, :, 0:DH].rearrange(
                                "p s a f -> p (s a) f"))
                    for wi in range(nw):
                        w = wc + wi
                        oh = ohp.tile([128, KW, 128], bf16, tag="oh")
                        nc.vector.tensor_tensor(
                            out=oh[:],
                            in0=iota_sb[:, None, :].to_broadcast([128, KW, 128]),
                            in1=segrel_sb[:, w * KW:(w + 1) * KW, None]
                                .to_broadcast([128, KW, 128]),
                            op=mybir.AluOpType.is_equal)
                        pw = psw.tile([128, 4 * DH], f32, tag="win")
                        for k in range(KW):
                            nc.tensor.matmul(
                                out=pw[:],
                                lhsT=oh[:, k, :],
                                rhs=msg[:, wi * KW + k, :, 0:DH],
                                start=(k == 0), stop=(k == KW - 1))
                        f0 = smp.tile([128, 2 * DH], f32, tag="f0")
                        f1 = smp.tile([128, 2 * DH], f32, tag="f1")
                        nc.scalar.activation(
                            out=f0[:], in_=pw[:, 0:2 * DH],
                            func=mybir.ActivationFunctionType.Copy)
                        nc.vector.tensor_tensor(
                            out=f1[:], in0=f0[:], in1=pw[:, 2 * DH:4 * DH],
                            op=mybir.AluOpType.add)
                        nc.vector.tensor_tensor(
                            out=agg[:, w, :], in0=f1[:, 0:DH], in1=f1[:, DH:2 * DH],
                            op=mybir.AluOpType.add)

                if debug and layer == 0:
                    nc.sync.dma_start(out=dbg_agg[:],
                                      in_=agg[:].rearrange("p t f -> p (t f)"))

                # ---- pre = dinv * (agg + hloc)   (reuse agg buffer)
                nc.vector.tensor_tensor(out=agg[:], in0=agg[:], in1=hloc[:],
                                        op=mybir.AluOpType.add)
                nc.vector.tensor_tensor(out=agg[:], in0=agg[:],
                                        in1=dinv_b([128, TN, DH]),
                                        op=mybir.AluOpType.mult)

                # ---- out_k = pre @ W + b  (layer 0: W already applied)
                if layer == 0:
                    nc.vector.tensor_tensor(
                        out=state[:], in0=agg[:],
                        in1=b0_sb[:, None, :].to_broadcast([128, TN, DH]),
                        op=mybir.AluOpType.add)
                else:
                    W_sb, b_sb, DO = ((w1_sb, b1_sb, DH) if layer == 1
                                      else (wf_sb, bf_sb, DC))
                    dst_buf = state if layer == 1 else final_sb
                    for t in range(TN):
                        ptp = pstr.tile([DH, 128], f32, tag="small", name=f"tr{t}")
                        nc.tensor.transpose(out=ptp[:], in_=agg[:, t, :],
                                            identity=ident[:])
                        preT = smp.tile([DH, 128], f32, tag="preT")
                        nc.scalar.activation(out=preT[:], in_=ptp[:],
                                             func=mybir.ActivationFunctionType.Copy)
                        po = pstr.tile([128, DO], f32, tag="small", name=f"po{t}")
                        nc.tensor.matmul(out=po[:], lhsT=preT[:], rhs=W_sb[:, 0:DO],
                                         start=True, stop=True)
                        nc.vector.tensor_tensor(
                            out=dst_buf[:, t, 0:DO], in0=po[:],
                            in1=b_sb[:, 0:DO],
                            op=mybir.AluOpType.add)

                if layer == 2:
                    nc.sync.dma_start(
                        out=out_d[0:(TN - 1) * 128, :].rearrange(
                            "(t p) f -> p t f", p=128),
                        in_=final_sb[:, 0:TN - 1, :])
                    lastn = NL - (TN - 1) * 128
                    nc.sync.dma_start(
                        out=out_d[(TN - 1) * 128: NL, :],
                        in_=final_sb[0:lastn, TN - 1, :])
                    continue

                # ---- pairnorm stats: colsums of state and state^2
                nc.vector.tensor_tensor(out=sq, in0=state[:], in1=state[:],
                                        op=mybir.AluOpType.mult)
                ps_s = psst.tile([DH, 1], f32, tag="st_s")
                ps_q = psst.tile([DH, 1], f32, tag="st_q")
                for t in range(TN):
                    kk = 128 if t < TN - 1 else (NL - (TN - 1) * 128)
                    nc.tensor.matmul(out=ps_s[:], lhsT=state[0:kk, t, :],
                                     rhs=ones_sb[0:kk, :],
                                     start=(t == 0), stop=(t == TN - 1))
                    nc.tensor.matmul(out=ps_q[:], lhsT=sq[0:kk, t, :],
                                     rhs=ones_sb[0:kk, :],
                                     start=(t == 0), stop=(t == TN - 1))
                s_sb = smp.tile([DH, 2], f32, tag="s_sb")
                nc.vector.tensor_copy(out=s_sb[:, 0:1], in_=ps_s[:])
                nc.vector.tensor_copy(out=s_sb[:, 1:2], in_=ps_q[:])
                nc.sync.dma_start(out=st_in.rearrange("o (p f) -> (o p) f", f=2),
                                  in_=s_sb[:])
                nc.gpsimd.collective_compute(
                    "AllReduce", mybir.AluOpType.add, replica_groups=rg,
                    ins=[st_in[:]], outs=[st_out[:]])
                nc.sync.dma_start(out=stats_sb[:], in_=st_out[:])
                # stats_sb[0, 2i] = colsum_i, stats_sb[0, 2i+1] = sqsum_i
                s_ap = stats_sb[:].rearrange("o (p f) -> o p f", f=2)  # [1,32,2]
                mean = smp.tile([1, DH], f32, tag="mean")
                nc.vector.tensor_scalar(
                    out=mean[:], in0=s_ap[:, :, 0], scalar1=1.0 / cfg.N,
                    scalar2=None, op0=mybir.AluOpType.mult)
                m2 = smp.tile([1, DH], f32, tag="m2")
                nc.vector.tensor_tensor(out=m2[:], in0=mean[:],
                                        in1=s_ap[:, :, 0],
                                        op=mybir.AluOpType.mult)  # s_f^2/N
                r1 = smp.tile([1, 1], f32, tag="r1")
                nc.vector.reduce_sum(out=r1[:], in_=m2[:],
                                     axis=mybir.AxisListType.X)
                qs = smp.tile([1, 1], f32, tag="qs")
                nc.vector.reduce_sum(out=qs[:], in_=s_ap[:, :, 1],
                                     axis=mybir.AxisListType.X)
                v_ = smp.tile([1, 1], f32, tag="v_")
                nc.vector.tensor_tensor(out=v_[:], in0=qs[:], in1=r1[:],
                                        op=mybir.AluOpType.subtract)
                nc.vector.tensor_scalar(
                    out=v_[:], in0=v_[:], scalar1=1.0 / cfg.N,
                    scalar2=cfg.EPS, op0=mybir.AluOpType.mult,
                    op1=mybir.AluOpType.add)
                rt = smp.tile([1, 1], f32, tag="rt")
                nc.scalar.activation(out=rt[:], in_=v_[:],
                                     func=mybir.ActivationFunctionType.Sqrt)
                scl = smp.tile([1, 1], f32, tag="scl")
                nc.vector.reciprocal(out=scl[:], in_=rt[:])

                # materialize mean|scale across partitions: ones-matmul bcast
                msc = smp.tile([1, DH + 1], f32, tag="msc")
                nc.vector.tensor_copy(out=msc[:, 0:DH], in_=mean[:])
                nc.vector.tensor_copy(out=msc[:, DH:DH + 1], in_=scl[:])
                pmsc = pstr.tile([128, DH + 1], f32, tag="small", name=f"pmsc{layer}")
                nc.tensor.matmul(out=pmsc[:], lhsT=ones_row[0:1, :],
                                 rhs=msc[:], start=True, stop=True)
                msc128 = smp.tile([128, DH + 1], f32, tag="msc128")
                nc.vector.tensor_copy(out=msc128[:], in_=pmsc[:])

                # ---- z = relu((state - mean) * scale); hloc = dinv * z
                nc.vector.tensor_tensor(
                    out=state[:], in0=state[:],
                    in1=msc128[:, None, 0:DH].to_broadcast([128, TN, DH]),
                    op=mybir.AluOpType.subtract)
                nc.vector.tensor_tensor(
                    out=state[:], in0=state[:],
                    in1=msc128[:, None, DH:DH + 1].to_broadcast([128, TN, DH]),
                    op=mybir.AluOpType.mult)
                nc.vector.tensor_scalar(
                    out=state[:], in0=state[:], scalar1=0.0, scalar2=None,
                    op0=mybir.AluOpType.max)
                nc.vector.tensor_tensor(out=hloc[:], in0=state[:],
                                        in1=dinv_b([128, TN, DH]),
                                        op=mybir.AluOpType.mult)
                write_table_and_allgather()

    nc.compile()
    return nc, names


# ----------------------------------------------------------------- entry

def make_inputs(cfg, pp, x, W0, b0, W1, b1, Wf, bf):
    """Per-core input maps for run_bass_kernel_spmd."""
    C, NL, NLP = cfg.C, cfg.NL, cfg.NLP
    x_pad = np.zeros((C, NLP, cfg.D_IN), dtype=BF16)
    for c in range(C):
        x_pad[c, :NL] = x[c * NL:(c + 1) * NL].astype(BF16)
    iota_row = np.tile(np.arange(128, dtype=np.float32).astype(BF16)[None, :], (128, 1))
    in_maps = []
    for c in range(C):
        in_maps.append({
            "x_bf": x_pad[c],
            "gidx": pp["gidx"][c],
            "segrel": pp["segrel"][c],
            "dinv_nm": pp["dinv_nm"][c],
            "w0": W0.astype(BF16),
            "w1": W1.astype(np.float32),
            "wf": Wf.astype(np.float32),
            "b0": np.tile(b0.reshape(1, -1).astype(np.float32), (128, 1)),
            "b1": np.tile(b1.reshape(1, -1).astype(np.float32), (128, 1)),
            "bf_": np.tile(bf.reshape(1, -1).astype(np.float32), (128, 1)),
            "iota_row": iota_row,
        })
    return in_maps


_CACHE = {}

def kernel(x, edge_index, W0, b0, W1, b1, Wf, bf):
    from concourse import bass_utils
    cfg = FULL
    x = np.asarray(x)
    edge_index = np.asarray(edge_index)
    pp = preprocess(cfg, edge_index)
    key = ("prog", pp["R"])
    if key not in _CACHE:
        _CACHE[key] = build_program(cfg, pp["R"])
    nc, _names = _CACHE[key]
    in_maps = make_inputs(cfg, pp, x, W0, b0, W1, b1, Wf, bf)
    res = bass_utils.run_bass_kernel_spmd(nc, in_maps, list(range(cfg.C)))
    out = np.concatenate([res.results[c]["out"] for c in range(cfg.C)], axis=0)
    return out.astype(np.float32)



# revision 18
# speedup vs baseline: 1.0702x; 1.0702x over previous
"""GCN node classifier on 8 Trainium2 NeuronCores.

3-layer GCN, N=100000 nodes, E=3.2M edges, d_in=512, d_h=32, n_cls=40.

Math refactor (linearity of gcn_conv):
    segsum(norm * (h@W)[src]) + b  ==  (dinv * segsum((dinv * h)[src])) @ W + b
so gathers move 32-wide rows, per-edge `norm` becomes node-level dinv
scaling, and the self-loop is a node-local term.

Sharding: nodes dst-sharded 12500/core. Each core aggregates all edges whose
dst lies in its shard, gathering source features from a replicated table
rebuilt per layer via AllGather.

Gather engine: batched gpsimd `dma_gather` (one instruction fetches
GROUP*R rows of 256B; SWDGE fixed cost ~1us amortized over ~8K rows vs the
128-row `indirect_dma_start` it replaces). int16 index constraint => the
source id space is split into 4 chunks of 32768; 256B-element constraint =>
the gather reads from a [N, 128] bf16 padded table (cols 0:32 valid)
expanded per layer from the compact AllGather output by strided DMA.

Scatter: per-edge one-hot segment matmuls on TensorE accumulating 128-dst
PSUM windows over (chunk, subtile), fixed R rows per (window, chunk) so the
SPMD program is identical on every core.
"""

import math
import numpy as np
import ml_dtypes

BF16 = ml_dtypes.bfloat16

# ---------------------------------------------------------------- config

class Cfg:
    def __init__(self, n_nodes, n_edges, d_in=512, d_h=32, n_cls=40, n_cores=8,
                 ch=32768, g=7):
        assert n_nodes % n_cores == 0
        self.N = n_nodes
        self.E = n_edges
        self.C = n_cores
        self.NL = n_nodes // n_cores          # nodes per core
        self.TN = math.ceil(self.NL / 128)    # dst windows per core
        self.NLP = self.TN * 128              # padded nodes per core
        self.D_IN = d_in
        self.DH = d_h
        self.DC = n_cls
        self.EPS = 1e-5
        self.CH = ch                          # src chunk size (int16 idx)
        self.NCH = math.ceil(n_nodes / self.CH)
        self.G = g                            # windows per gather call
        assert self.TN % self.G == 0
        self.NG = self.TN // self.G           # gather groups

FULL = Cfg(100000, 3200000)

# ------------------------------------------------------- host preprocessing

def preprocess(cfg, edge_index):
    """Vectorized build of per-core gather indices + segment labels.

    Row layout per core (one 'call' = one dma_gather = (group g, chunk c)):
      per group g: chunks c=0..NCH-1, each G windows x R_c rows;
      gathered row i of a call -> out[i%128, i//128, :].

    Returns dict with RS (per-chunk padded rows per window) plus per-core:
      gidx  [C, 128, NG*sum(G*R_c/16)] int16  (16-wrapped, replicated)
      segrel[C, 128, NG*sum(G*R_c/128)] bf16  (dst_rel per row, -1 pad)
      dinv_nm [C, 128, TN] f32
    """
    N, C, NL, TN, CH, NCH, G = (cfg.N, cfg.C, cfg.NL, cfg.TN, cfg.CH,
                                cfg.NCH, cfg.G)
    src = np.asarray(edge_index[0], dtype=np.int64)
    dst = np.asarray(edge_index[1], dtype=np.int64)
    deg = np.bincount(dst, minlength=N).astype(np.float64) + 1.0  # + self loop
    dinv = (1.0 / np.sqrt(deg)).astype(np.float32)

    core = dst // NL
    nl = dst - core * NL
    w = nl >> 7                       # window in core
    dst_rel = nl & 127
    ch = src // CH
    src_rel = (src - ch * CH).astype(np.int64)

    # group id per (core, window, chunk)
    gid = (core * TN + w) * NCH + ch
    n_gid = C * TN * NCH
    cnt = np.bincount(gid, minlength=n_gid)
    # per-chunk padded rows-per-window
    cell = cnt.reshape(C, TN, NCH)
    RS = [int(cell[:, :, c].max()) for c in range(NCH)]
    RS = [max(128, ((r + 127) // 128) * 128) for r in RS]
    SR = sum(RS)
    roff = np.concatenate([[0], np.cumsum([G * r for r in RS])])  # in-group

    order = np.argsort(gid, kind="stable")
    gid_s = gid[order]
    starts = np.zeros(n_gid + 1, dtype=np.int64)
    np.cumsum(cnt, out=starts[1:])
    rank = np.arange(len(order), dtype=np.int64) - starts[gid_s]

    # flat position inside the per-core row stream
    core_s = gid_s // (TN * NCH)
    rem = gid_s - core_s * (TN * NCH)
    w_s = rem // NCH
    c_s = rem - w_s * NCH
    g_s = w_s // G
    wi_s = w_s - g_s * G
    RS_a = np.asarray(RS, dtype=np.int64)
    pos = (g_s * (G * SR) + roff[c_s] + wi_s * RS_a[c_s] + rank)

    rows_per_core = cfg.NG * G * SR
    idx_flat = np.zeros((C, rows_per_core), dtype=np.int16)
    seg_flat = np.full((C, rows_per_core), -1.0, dtype=np.float32)
    idx_flat[core_s, pos] = src_rel[order].astype(np.int16)
    seg_flat[core_s, pos] = dst_rel[order]

    # wrap indices per call: rows (G*R_c) -> [cols, 16] -> [16, cols]
    idx3 = idx_flat.reshape(C, cfg.NG, G * SR)
    seg3 = seg_flat.reshape(C, cfg.NG, G * SR)
    gxs, sgs = [], []
    for c in range(NCH):
        blk = idx3[:, :, int(roff[c]):int(roff[c + 1])]      # [C, NG, G*R_c]
        gxs.append(blk.reshape(C, cfg.NG, G * RS[c] // 16, 16)
                   .transpose(0, 3, 1, 2)
                   .reshape(C, 16, cfg.NG, G * RS[c] // 16))
        sblk = seg3[:, :, int(roff[c]):int(roff[c + 1])]
        sgs.append(sblk.reshape(C, cfg.NG, G * RS[c] // 128, 128)
                   .transpose(0, 3, 1, 2)
                   .reshape(C, 128, cfg.NG, G * RS[c] // 128))
    # interleave per group: [g][c] order along the col axis
    gidx = np.concatenate(gxs, axis=3).reshape(C, 16, -1)
    gidx = np.tile(gidx, (1, 8, 1))           # replicate to 128 partitions
    segrel = np.concatenate(sgs, axis=3).reshape(C, 128, -1)

    dinv_nm = np.zeros((C, 128, TN), dtype=np.float32)
    for c in range(C):
        v = c * NL + np.arange(cfg.NLP)
        valid = v < (c + 1) * NL
        dd = np.where(valid, dinv[np.minimum(v, N - 1)], 0.0)
        dinv_nm[c] = dd.reshape(TN, 128).T
    return dict(RS=tuple(RS), gidx=np.ascontiguousarray(gidx),
                segrel=np.ascontiguousarray(segrel.astype(BF16)),
                dinv_nm=dinv_nm)


# ------------------------------------------------------------ device program

def build_program(cfg, RS, debug=False):
    """Build the SPMD bass program. Returns (nc, names)."""
    from concourse import bass, bacc, mybir, tile
    from concourse.masks import make_identity

    f32 = mybir.dt.float32
    bf16 = mybir.dt.bfloat16
    i16 = mybir.dt.int16

    N, NL, NLP, TN, D_IN, DH, DC = (cfg.N, cfg.NL, cfg.NLP, cfg.TN,
                                    cfg.D_IN, cfg.DH, cfg.DC)
    CH, NCH, G, NG = cfg.CH, cfg.NCH, cfg.G, cfg.NG
    KWS = [r // 128 for r in RS]          # subtiles per (window, chunk)
    NIS = [G * r for r in RS]             # idxs per gather call, by chunk
    ICS = [ni // 16 for ni in NIS]        # idx cols per call, by chunk
    MCS = [G * kw for kw in KWS]          # msg cols per call, by chunk
    SIC = sum(ICS)                        # idx cols per group
    SMC = sum(MCS)                        # msg cols per group
    ioff = [0]
    moff = [0]
    for c in range(NCH):
        ioff.append(ioff[-1] + ICS[c])
        moff.append(moff[-1] + MCS[c])
    KC = D_IN // 128              # contraction chunks for x@W0
    rg = [list(range(cfg.C))]
    chsz = [min(N, (c + 1) * CH) - c * CH for c in range(NCH)]

    nc = bacc.Bacc("TRN2", target_bir_lowering=False, debug=False,
                   num_devices=cfg.C)

    # -------- kernel I/O
    x_bf = nc.dram_tensor("x_bf", [NLP, D_IN], bf16, kind="ExternalInput").ap()
    gidx_d = nc.dram_tensor("gidx", [128, NG * SIC], i16,
                            kind="ExternalInput").ap()
    segrel_d = nc.dram_tensor("segrel", [128, NG * SMC], bf16,
                              kind="ExternalInput").ap()
    dinv_d = nc.dram_tensor("dinv_nm", [128, TN], f32, kind="ExternalInput").ap()
    w0_d = nc.dram_tensor("w0", [D_IN, DH], bf16, kind="ExternalInput").ap()
    w1_d = nc.dram_tensor("w1", [DH, DH], f32, kind="ExternalInput").ap()
    wf_d = nc.dram_tensor("wf", [DH, DC], f32, kind="ExternalInput").ap()
    b0_d = nc.dram_tensor("b0", [128, DH], f32, kind="ExternalInput").ap()
    b1_d = nc.dram_tensor("b1", [128, DH], f32, kind="ExternalInput").ap()
    bf_d = nc.dram_tensor("bf_", [128, DC], f32, kind="ExternalInput").ap()
    iota_d = nc.dram_tensor("iota_row", [128, 128], bf16, kind="ExternalInput").ap()
    out_d = nc.dram_tensor("out", [NL, DC], f32, kind="ExternalOutput").ap()

    # -------- internal DRAM
    table = nc.dram_tensor("h_table", [N, DH], bf16).ap()       # compact
    table_p = nc.dram_tensor("h_table_p", [N, 128], bf16).ap()  # padded rows
    shard = nc.dram_tensor("h_shard", [NLP, DH], bf16).ap()
    st_in = nc.dram_tensor("st_in", [1, 64], f32).ap()
    st_out = nc.dram_tensor("st_out", [1, 64], f32).ap()

    names = ["x_bf", "gidx", "segrel", "dinv_nm", "w0", "w1", "wf",
             "b0", "b1", "bf_", "iota_row"]

    with tile.TileContext(nc) as tc:
        import contextlib
        with contextlib.ExitStack() as ctx:
            big = ctx.enter_context(tc.tile_pool(name="big", bufs=1))
            gp = ctx.enter_context(tc.tile_pool(name="gp", bufs=2))
            msgp = ctx.enter_context(tc.tile_pool(name="msg", bufs=2))
            ohp = ctx.enter_context(tc.tile_pool(name="oh", bufs=2))
            smp = ctx.enter_context(tc.tile_pool(name="sm", bufs=4))
            psw = ctx.enter_context(tc.tile_pool(name="psw", bufs=2, space="PSUM"))
            pstr = ctx.enter_context(tc.tile_pool(name="pstr", bufs=2, space="PSUM"))
            psst = ctx.enter_context(tc.tile_pool(name="psst", bufs=1, space="PSUM"))
            psA = ctx.enter_context(tc.tile_pool(name="psA", bufs=2, space="PSUM"))

            # ---- residents
            segrel_sb = big.tile([128, NG * SMC], bf16)
            dinv_sb = big.tile([128, TN], f32)
            w0_sb = big.tile([128, D_IN // 128, DH], bf16)
            w1_sb = big.tile([DH, DH], f32)
            wf_sb = big.tile([DH, DC], f32)
            b0_sb = big.tile([128, DH], f32)
            b1_sb = big.tile([128, DH], f32)
            bf_sb = big.tile([128, DC], f32)
            iota_sb = big.tile([128, 128], bf16)
            ident = big.tile([128, 128], f32)
            ones_sb = big.tile([128, 1], f32)
            ones_row = big.tile([1, 128], f32)
            stats_sb = big.tile([1, 64], f32)

            hloc = big.tile([128, TN, DH], f32)     # dinv * h (self term)
            agg = big.tile([128, TN, DH], f32)
            state = big.tile([128, TN, DH], f32)
            shard_sb = big.tile([128, TN, DH], bf16)
            final_sb = big.tile([128, TN, DC], f32)
            sq = final_sb[:, :, 0:DH]               # scratch; real use layer 2

            nc.sync.dma_start(out=segrel_sb[:], in_=segrel_d[:])
            nc.sync.dma_start(out=dinv_sb[:], in_=dinv_d[:])
            nc.sync.dma_start(out=w0_sb[:], in_=w0_d.rearrange("(c p) f -> p c f", p=128))
            nc.sync.dma_start(out=w1_sb[:], in_=w1_d[:])
            nc.sync.dma_start(out=wf_sb[:], in_=wf_d[:])
            nc.sync.dma_start(out=b0_sb[:], in_=b0_d[:])
            nc.sync.dma_start(out=b1_sb[:], in_=b1_d[:])
            nc.sync.dma_start(out=bf_sb[:], in_=bf_d[:])
            nc.sync.dma_start(out=iota_sb[:], in_=iota_d[:])
            make_identity(nc, ident[:])
            nc.vector.memset(ones_sb[:], 1.0)
            nc.vector.memset(ones_row[:], 1.0)

            def dinv_b(ap_shape):
                return dinv_sb[:, :, None].to_broadcast(ap_shape)

            # ---------------- phase A: t0 = x @ W0 (bf16), hloc = dinv*t0
            NB = 4
            TH0 = (TN + NB - 1) // NB
            bands = [(b * TH0, min(TN, (b + 1) * TH0)) for b in range(NB)]
            with tc.tile_pool(name="xTp", bufs=1) as xTp:
                for h, (tlo, thi) in enumerate(bands):
                    nh = thi - tlo
                    if nh <= 0:
                        continue
                    xT = [xTp.tile([128, TH0 * 128], bf16, tag=f"xT{c}",
                                   name=f"xT{c}_{h}") for c in range(KC)]
                    for c in range(KC):
                        nc.sync.dma_start_transpose(
                            out=xT[c][:, 0:nh * 128],
                            in_=x_bf[tlo * 128: thi * 128,
                                     128 * c:128 * (c + 1)])
                    for t in range(tlo, thi):
                        t0 = psA.tile([128, DH], f32, tag="psA", name=f"t0_{t}")
                        for c in range(KC):
                            nc.tensor.matmul(
                                out=t0[:],
                                lhsT=xT[c][:, 128 * (t - tlo):128 * (t - tlo + 1)],
                                rhs=w0_sb[:, c, :],
                                start=(c == 0), stop=(c == KC - 1))
                        nc.vector.tensor_tensor(
                            out=hloc[:, t, :], in0=t0[:],
                            in1=dinv_sb[:, t:t + 1].to_broadcast([128, DH]),
                            op=mybir.AluOpType.mult)

            def write_table_allgather_expand():
                nc.vector.tensor_copy(out=shard_sb[:], in_=hloc[:])
                nc.sync.dma_start(
                    out=shard.rearrange("(t p) f -> p t f", p=128),
                    in_=shard_sb[:])
                nc.gpsimd.collective_compute(
                    "AllGather", mybir.AluOpType.bypass,
                    replica_groups=rg,
                    ins=[shard[0:NL, :]],
                    outs=[table[0:N, :]],
                )
                for c in range(NCH):
                    c0 = c * CH
                    nc.sync.dma_start(
                        out=table_p[c0:c0 + chsz[c], 0:DH],
                        in_=table[c0:c0 + chsz[c], :])

            write_table_allgather_expand()

            # ---------------- layers
            for layer in range(3):
                # ---- gather + segment-sum into agg
                for g in range(NG):
                    gt = gp.tile([128, SIC], i16, tag="gidx")
                    nc.sync.dma_start(
                        out=gt[:],
                        in_=gidx_d[:, g * SIC:(g + 1) * SIC])
                    aggsl = agg[:, g * G:(g + 1) * G, :]
                    for c in range(NCH):
                        mc = MCS[c]
                        msg = msgp.tile([128, mc, 128], bf16, tag=f"msg{mc}")
                        nc.gpsimd.dma_gather(
                            out_ap=msg[:],
                            in_ap=table_p[c * CH:c * CH + chsz[c], :],
                            idxs_ap=gt[:, ioff[c]:ioff[c + 1]],
                            num_idxs=NIS[c],
                            num_idxs_reg=NIS[c],
                            elem_size=128,
                            single_packet=False,
                        )
                        oh = ohp.tile([128, mc, 128], bf16, tag=f"oh{mc}")
                        sc0 = g * SMC + moff[c]
                        nc.vector.tensor_tensor(
                            out=oh[:],
                            in0=iota_sb[:, None, :].to_broadcast([128, mc, 128]),
                            in1=segrel_sb[:, sc0:sc0 + mc, None]
                                .to_broadcast([128, mc, 128]),
                            op=mybir.AluOpType.is_equal)
                        pws = psw.tile([128, G, DH], f32, tag="pw",
                                       name=f"pw_{layer}_{g}_{c}")
                        for wi in range(G):
                            for k in range(KWS[c]):
                                s = wi * KWS[c] + k
                                nc.tensor.matmul(
                                    out=pws[:, wi, :],
                                    lhsT=oh[:, s, :],
                                    rhs=msg[:, s, 0:DH],
                                    start=(k == 0),
                                    stop=(k == KWS[c] - 1))
                        if c == 0:
                            nc.vector.tensor_copy(out=aggsl, in_=pws[:])
                        else:
                            nc.vector.tensor_tensor(
                                out=aggsl, in0=aggsl, in1=pws[:],
                                op=mybir.AluOpType.add)

                # ---- pre = dinv * (agg + hloc)   (reuse agg buffer)
                nc.vector.tensor_tensor(out=agg[:], in0=agg[:], in1=hloc[:],
                                        op=mybir.AluOpType.add)
                nc.vector.tensor_tensor(out=agg[:], in0=agg[:],
                                        in1=dinv_b([128, TN, DH]),
                                        op=mybir.AluOpType.mult)

                # ---- out_k = pre @ W + b  (layer 0: W already applied)
                if layer == 0:
                    nc.vector.tensor_tensor(
                        out=state[:], in0=agg[:],
                        in1=b0_sb[:, None, :].to_broadcast([128, TN, DH]),
                        op=mybir.AluOpType.add)
                else:
                    W_sb, b_sb, DO = ((w1_sb, b1_sb, DH) if layer == 1
                                      else (wf_sb, bf_sb, DC))
                    dst_buf = state if layer == 1 else final_sb
                    for t in range(TN):
                        ptp = pstr.tile([DH, 128], f32, tag="small", name=f"tr{t}")
                        nc.tensor.transpose(out=ptp[:], in_=agg[:, t, :],
                                            identity=ident[:])
                        preT = smp.tile([DH, 128], f32, tag="preT")
                        nc.scalar.activation(out=preT[:], in_=ptp[:],
                                             func=mybir.ActivationFunctionType.Copy)
                        po = pstr.tile([128, DO], f32, tag="small", name=f"po{t}")
                        nc.tensor.matmul(out=po[:], lhsT=preT[:], rhs=W_sb[:, 0:DO],
                                         start=True, stop=True)
                        nc.vector.tensor_tensor(
                            out=dst_buf[:, t, 0:DO], in0=po[:],
                            in1=b_sb[:, 0:DO],
                            op=mybir.AluOpType.add)

                if layer == 2:
                    nc.sync.dma_start(
                        out=out_d[0:(TN - 1) * 128, :].rearrange(
                            "(t p) f -> p t f", p=128),
                        in_=final_sb[:, 0:TN - 1, :])
                    lastn = NL - (TN - 1) * 128
                    nc.sync.dma_start(
                        out=out_d[(TN - 1) * 128: NL, :],
                        in_=final_sb[0:lastn, TN - 1, :])
                    continue

                # ---- pairnorm stats: colsums of state and state^2
                nc.vector.tensor_tensor(out=sq, in0=state[:], in1=state[:],
                                        op=mybir.AluOpType.mult)
                ps_s = psst.tile([DH, 1], f32, tag="st_s")
                ps_q = psst.tile([DH, 1], f32, tag="st_q")
                for t in range(TN):
                    kk = 128 if t < TN - 1 else (NL - (TN - 1) * 128)
                    nc.tensor.matmul(out=ps_s[:], lhsT=state[0:kk, t, :],
                                     rhs=ones_sb[0:kk, :],
                                     start=(t == 0), stop=(t == TN - 1))
                    nc.tensor.matmul(out=ps_q[:], lhsT=sq[0:kk, t, :],
                                     rhs=ones_sb[0:kk, :],
                                     start=(t == 0), stop=(t == TN - 1))
                s_sb = smp.tile([DH, 2], f32, tag="s_sb")
                nc.vector.tensor_copy(out=s_sb[:, 0:1], in_=ps_s[:])
                nc.vector.tensor_copy(out=s_sb[:, 1:2], in_=ps_q[:])
                nc.sync.dma_start(out=st_in.rearrange("o (p f) -> (o p) f", f=2),
                                  in_=s_sb[:])
                nc.gpsimd.collective_compute(
                    "AllReduce", mybir.AluOpType.add, replica_groups=rg,
                    ins=[st_in[:]], outs=[st_out[:]])
                nc.sync.dma_start(out=stats_sb[:], in_=st_out[:])
                # stats_sb[0, 2i] = colsum_i, stats_sb[0, 2i+1] = sqsum_i
                s_ap = stats_sb[:].rearrange("o (p f) -> o p f", f=2)  # [1,32,2]
                mean = smp.tile([1, DH], f32, tag="mean")
                nc.vector.tensor_scalar(
                    out=mean[:], in0=s_ap[:, :, 0], scalar1=1.0 / cfg.N,
                    scalar2=None, op0=mybir.AluOpType.mult)
                m2 = smp.tile([1, DH], f32, tag="m2")
                nc.vector.tensor_tensor(out=m2[:], in0=mean[:],
                                        in1=s_ap[:, :, 0],
                                        op=mybir.AluOpType.mult)  # s_f^2/N
                r1 = smp.tile([1, 1], f32, tag="r1")
                nc.vector.reduce_sum(out=r1[:], in_=m2[:],
                                     axis=mybir.AxisListType.X)
                qs = smp.tile([1, 1], f32, tag="qs")
                nc.vector.reduce_sum(out=qs[:], in_=s_ap[:, :, 1],
                                     axis=mybir.AxisListType.X)
                v_ = smp.tile([1, 1], f32, tag="v_")
                nc.vector.tensor_tensor(out=v_[:], in0=qs[:], in1=r1[:],
                                        op=mybir.AluOpType.subtract)
                nc.vector.tensor_scalar(
                    out=v_[:], in0=v_[:], scalar1=1.0 / cfg.N,
                    scalar2=cfg.EPS, op0=mybir.AluOpType.mult,
                    op1=mybir.AluOpType.add)
                rt = smp.tile([1, 1], f32, tag="rt")
                nc.scalar.activation(out=rt[:], in_=v_[:],
                                     func=mybir.ActivationFunctionType.Sqrt)
                scl = smp.tile([1, 1], f32, tag="scl")
                nc.vector.reciprocal(out=scl[:], in_=rt[:])

                # materialize mean|scale across partitions: ones-matmul bcast
                msc = smp.tile([1, DH + 1], f32, tag="msc")
                nc.vector.tensor_copy(out=msc[:, 0:DH], in_=mean[:])
                nc.vector.tensor_copy(out=msc[:, DH:DH + 1], in_=scl[:])
                pmsc = pstr.tile([128, DH + 1], f32, tag="small", name=f"pmsc{layer}")
                nc.tensor.matmul(out=pmsc[:], lhsT=ones_row[0:1, :],
                                 rhs=msc[:], start=True, stop=True)
                msc128 = smp.tile([128, DH + 1], f32, tag="msc128")
                nc.vector.tensor_copy(out=msc128[:], in_=pmsc[:])

                # ---- z = relu((state - mean) * scale); hloc = dinv * z
                nc.vector.tensor_tensor(
                    out=state[:], in0=state[:],
                    in1=msc128[:, None, 0:DH].to_broadcast([128, TN, DH]),
                    op=mybir.AluOpType.subtract)
                nc.vector.tensor_tensor(
                    out=state[:], in0=state[:],
                    in1=msc128[:, None, DH:DH + 1].to_broadcast([128, TN, DH]),
                    op=mybir.AluOpType.mult)
                nc.vector.tensor_scalar(
                    out=state[:], in0=state[:], scalar1=0.0, scalar2=None,
                    op0=mybir.AluOpType.max)
                nc.vector.tensor_tensor(out=hloc[:], in0=state[:],
                                        in1=dinv_b([128, TN, DH]),
                                        op=mybir.AluOpType.mult)
                write_table_allgather_expand()

    nc.compile()
    return nc, names


# ----------------------------------------------------------------- entry

def make_inputs(cfg, pp, x, W0, b0, W1, b1, Wf, bf):
    """Per-core input maps for run_bass_kernel_spmd."""
    C, NL, NLP = cfg.C, cfg.NL, cfg.NLP
    x_pad = np.zeros((C, NLP, cfg.D_IN), dtype=BF16)
    for c in range(C):
        x_pad[c, :NL] = x[c * NL:(c + 1) * NL].astype(BF16)
    iota_row = np.tile(np.arange(128, dtype=np.float32).astype(BF16)[None, :], (128, 1))
    in_maps = []
    for c in range(C):
        in_maps.append({
            "x_bf": x_pad[c],
            "gidx": pp["gidx"][c],
            "segrel": pp["segrel"][c],
            "dinv_nm": pp["dinv_nm"][c],
            "w0": W0.astype(BF16),
            "w1": W1.astype(np.float32),
            "wf": Wf.astype(np.float32),
            "b0": np.tile(b0.reshape(1, -1).astype(np.float32), (128, 1)),
            "b1": np.tile(b1.reshape(1, -1).astype(np.float32), (128, 1)),
            "bf_": np.tile(bf.reshape(1, -1).astype(np.float32), (128, 1)),
            "iota_row": iota_row,
        })
    return in_maps


_CACHE = {}

def kernel(x, edge_index, W0, b0, W1, b1, Wf, bf):
    from concourse import bass_utils
    cfg = FULL
    x = np.asarray(x)
    edge_index = np.asarray(edge_index)
    pp = preprocess(cfg, edge_index)
    key = ("prog", pp["RS"])
    if key not in _CACHE:
        _CACHE[key] = build_program(cfg, pp["RS"])
    nc, _names = _CACHE[key]
    in_maps = make_inputs(cfg, pp, x, W0, b0, W1, b1, Wf, bf)
    res = bass_utils.run_bass_kernel_spmd(nc, in_maps, list(range(cfg.C)))
    out = np.concatenate([res.results[c]["out"] for c in range(cfg.C)], axis=0)
    return out.astype(np.float32)


# revision 19
# speedup vs baseline: 1.2191x; 1.1391x over previous
"""GCN node classifier on 8 Trainium2 NeuronCores.

3-layer GCN, N=100000 nodes, E=3.2M edges, d_in=512, d_h=32, n_cls=40.

Math refactor (linearity of gcn_conv):
    segsum(norm * (h@W)[src]) + b  ==  (dinv * segsum((dinv * h)[src])) @ W + b
so gathers move 32-wide rows, per-edge `norm` becomes node-level dinv
scaling, and the self-loop is a node-local term.

Sharding: nodes dst-sharded 12500/core. Each core aggregates all edges whose
dst lies in its shard, gathering source features from a replicated table
rebuilt per layer via AllGather.

Gather engine: batched gpsimd `dma_gather` (one instruction fetches
GROUP*R rows of 256B; SWDGE fixed cost ~1us amortized over ~8K rows vs the
128-row `indirect_dma_start` it replaces). int16 index constraint => the
source id space is split into 4 chunks of 32768; 256B-element constraint =>
the gather reads from a [N, 128] bf16 padded table (cols 0:32 valid)
expanded per layer from the compact AllGather output by strided DMA.

Scatter: per-edge one-hot segment matmuls on TensorE accumulating 128-dst
PSUM windows over (chunk, subtile), fixed R rows per (window, chunk) so the
SPMD program is identical on every core.
"""

import math
import numpy as np
import ml_dtypes

BF16 = ml_dtypes.bfloat16

# ---------------------------------------------------------------- config

class Cfg:
    def __init__(self, n_nodes, n_edges, d_in=512, d_h=32, n_cls=40, n_cores=8,
                 ch=32768, g=7):
        assert n_nodes % n_cores == 0
        self.N = n_nodes
        self.E = n_edges
        self.C = n_cores
        self.NL = n_nodes // n_cores          # nodes per core
        self.TN = math.ceil(self.NL / 128)    # dst windows per core
        self.NLP = self.TN * 128              # padded nodes per core
        self.D_IN = d_in
        self.DH = d_h
        self.DC = n_cls
        self.EPS = 1e-5
        self.CH = ch                          # src chunk size (int16 idx)
        self.NCH = math.ceil(n_nodes / self.CH)
        self.G = g                            # windows per gather call
        assert self.TN % self.G == 0
        self.NG = self.TN // self.G           # gather groups

FULL = Cfg(100000, 3200000, g=2)

# ------------------------------------------------------- host preprocessing

def preprocess(cfg, edge_index):
    """Vectorized build of per-core gather indices + segment labels.

    Row layout per core (one 'call' = one dma_gather = (group g, chunk c)):
      per group g: chunks c=0..NCH-1, each G windows x R_c rows;
      gathered row i of a call -> out[i%128, i//128, :].

    Returns dict with RS (per-chunk padded rows per window) plus per-core:
      gidx  [C, 128, NG*sum(G*R_c/16)] int16  (16-wrapped, replicated)
      segrel[C, 128, NG*sum(G*R_c/128)] bf16  (dst_rel per row, -1 pad)
      dinv_nm [C, 128, TN] f32
    """
    N, C, NL, TN, CH, NCH, G = (cfg.N, cfg.C, cfg.NL, cfg.TN, cfg.CH,
                                cfg.NCH, cfg.G)
    src = np.asarray(edge_index[0], dtype=np.int64)
    dst = np.asarray(edge_index[1], dtype=np.int64)
    deg = np.bincount(dst, minlength=N).astype(np.float64) + 1.0  # + self loop
    dinv = (1.0 / np.sqrt(deg)).astype(np.float32)

    core = dst // NL
    nl = dst - core * NL
    w = nl >> 7                       # window in core
    dst_rel = nl & 127
    ch = src // CH
    src_rel = (src - ch * CH).astype(np.int64)

    # group id per (core, window, chunk)
    gid = (core * TN + w) * NCH + ch
    n_gid = C * TN * NCH
    cnt = np.bincount(gid, minlength=n_gid)
    # per-chunk padded rows-per-window
    cell = cnt.reshape(C, TN, NCH)
    RS = [int(cell[:, :, c].max()) for c in range(NCH)]
    RS = [max(128, ((r + 127) // 128) * 128) for r in RS]
    SR = sum(RS)
    roff = np.concatenate([[0], np.cumsum([G * r for r in RS])])  # in-group

    order = np.argsort(gid, kind="stable")
    gid_s = gid[order]
    starts = np.zeros(n_gid + 1, dtype=np.int64)
    np.cumsum(cnt, out=starts[1:])
    rank = np.arange(len(order), dtype=np.int64) - starts[gid_s]

    # flat position inside the per-core row stream
    core_s = gid_s // (TN * NCH)
    rem = gid_s - core_s * (TN * NCH)
    w_s = rem // NCH
    c_s = rem - w_s * NCH
    g_s = w_s // G
    wi_s = w_s - g_s * G
    RS_a = np.asarray(RS, dtype=np.int64)
    pos = (g_s * (G * SR) + roff[c_s] + wi_s * RS_a[c_s] + rank)

    rows_per_core = cfg.NG * G * SR
    idx_flat = np.zeros((C, rows_per_core), dtype=np.int16)
    seg_flat = np.full((C, rows_per_core), -1.0, dtype=np.float32)
    idx_flat[core_s, pos] = src_rel[order].astype(np.int16)
    seg_flat[core_s, pos] = dst_rel[order]

    # wrap indices per call: rows (G*R_c) -> [cols, 16] -> [16, cols]
    idx3 = idx_flat.reshape(C, cfg.NG, G * SR)
    seg3 = seg_flat.reshape(C, cfg.NG, G * SR)
    gxs, sgs = [], []
    for c in range(NCH):
        blk = idx3[:, :, int(roff[c]):int(roff[c + 1])]      # [C, NG, G*R_c]
        gxs.append(blk.reshape(C, cfg.NG, G * RS[c] // 16, 16)
                   .transpose(0, 3, 1, 2)
                   .reshape(C, 16, cfg.NG, G * RS[c] // 16))
        sblk = seg3[:, :, int(roff[c]):int(roff[c + 1])]
        sgs.append(sblk.reshape(C, cfg.NG, G * RS[c] // 128, 128)
                   .transpose(0, 3, 1, 2)
                   .reshape(C, 128, cfg.NG, G * RS[c] // 128))
    # interleave per group: [g][c] order along the col axis
    gidx = np.concatenate(gxs, axis=3).reshape(C, 16, -1)
    gidx = np.tile(gidx, (1, 8, 1))           # replicate to 128 partitions
    segrel = np.concatenate(sgs, axis=3).reshape(C, 128, -1)

    dinv_nm = np.zeros((C, 128, TN), dtype=np.float32)
    for c in range(C):
        v = c * NL + np.arange(cfg.NLP)
        valid = v < (c + 1) * NL
        dd = np.where(valid, dinv[np.minimum(v, N - 1)], 0.0)
        dinv_nm[c] = dd.reshape(TN, 128).T
    return dict(RS=tuple(RS), gidx=np.ascontiguousarray(gidx),
                segrel=np.ascontiguousarray(segrel.astype(BF16)),
                dinv_nm=dinv_nm)


# ------------------------------------------------------------ device program

def build_program(cfg, RS, debug=False):
    """Build the SPMD bass program. Returns (nc, names)."""
    from concourse import bass, bacc, mybir, tile
    from concourse.masks import make_identity

    f32 = mybir.dt.float32
    bf16 = mybir.dt.bfloat16
    i16 = mybir.dt.int16

    N, NL, NLP, TN, D_IN, DH, DC = (cfg.N, cfg.NL, cfg.NLP, cfg.TN,
                                    cfg.D_IN, cfg.DH, cfg.DC)
    CH, NCH, G, NG = cfg.CH, cfg.NCH, cfg.G, cfg.NG
    KWS = [r // 128 for r in RS]          # subtiles per (window, chunk)
    NIS = [G * r for r in RS]             # idxs per gather call, by chunk
    ICS = [ni // 16 for ni in NIS]        # idx cols per call, by chunk
    MCS = [G * kw for kw in KWS]          # msg cols per call, by chunk
    SIC = sum(ICS)                        # idx cols per group
    SMC = sum(MCS)                        # msg cols per group
    ioff = [0]
    moff = [0]
    for c in range(NCH):
        ioff.append(ioff[-1] + ICS[c])
        moff.append(moff[-1] + MCS[c])
    KC = D_IN // 128              # contraction chunks for x@W0
    rg = [list(range(cfg.C))]
    chsz = [min(N, (c + 1) * CH) - c * CH for c in range(NCH)]

    nc = bacc.Bacc("TRN2", target_bir_lowering=False, debug=False,
                   num_devices=cfg.C, num_swdge_queues=4)

    # -------- kernel I/O
    x_bf = nc.dram_tensor("x_bf", [NLP, D_IN], bf16, kind="ExternalInput").ap()
    gidx_d = nc.dram_tensor("gidx", [128, NG * SIC], i16,
                            kind="ExternalInput").ap()
    segrel_d = nc.dram_tensor("segrel", [128, NG * SMC], bf16,
                              kind="ExternalInput").ap()
    dinv_d = nc.dram_tensor("dinv_nm", [128, TN], f32, kind="ExternalInput").ap()
    w0_d = nc.dram_tensor("w0", [D_IN, DH], bf16, kind="ExternalInput").ap()
    w1_d = nc.dram_tensor("w1", [DH, DH], f32, kind="ExternalInput").ap()
    wf_d = nc.dram_tensor("wf", [DH, DC], f32, kind="ExternalInput").ap()
    b0_d = nc.dram_tensor("b0", [128, DH], f32, kind="ExternalInput").ap()
    b1_d = nc.dram_tensor("b1", [128, DH], f32, kind="ExternalInput").ap()
    bf_d = nc.dram_tensor("bf_", [128, DC], f32, kind="ExternalInput").ap()
    iota_d = nc.dram_tensor("iota_row", [128, 128], bf16, kind="ExternalInput").ap()
    out_d = nc.dram_tensor("out", [NL, DC], f32, kind="ExternalOutput").ap()

    # -------- internal DRAM
    table = nc.dram_tensor("h_table", [N, DH], bf16).ap()       # compact
    table_p = nc.dram_tensor("h_table_p", [N, 128], bf16).ap()  # padded rows
    shard = nc.dram_tensor("h_shard", [NLP, DH], bf16).ap()
    st_in = nc.dram_tensor("st_in", [1, 64], f32).ap()
    st_out = nc.dram_tensor("st_out", [1, 64], f32).ap()

    names = ["x_bf", "gidx", "segrel", "dinv_nm", "w0", "w1", "wf",
             "b0", "b1", "bf_", "iota_row"]

    with tile.TileContext(nc) as tc:
        import contextlib
        with contextlib.ExitStack() as ctx:
            big = ctx.enter_context(tc.tile_pool(name="big", bufs=1))
            gp = ctx.enter_context(tc.tile_pool(name="gp", bufs=3))
            msgp = ctx.enter_context(tc.tile_pool(name="msg", bufs=8))
            ohp = ctx.enter_context(tc.tile_pool(name="oh", bufs=4))
            smp = ctx.enter_context(tc.tile_pool(name="sm", bufs=4))
            psw = ctx.enter_context(tc.tile_pool(name="psw", bufs=2, space="PSUM"))
            pstr = ctx.enter_context(tc.tile_pool(name="pstr", bufs=2, space="PSUM"))
            psst = ctx.enter_context(tc.tile_pool(name="psst", bufs=1, space="PSUM"))
            psA = ctx.enter_context(tc.tile_pool(name="psA", bufs=2, space="PSUM"))

            # ---- residents
            segrel_sb = big.tile([128, NG * SMC], bf16)
            dinv_sb = big.tile([128, TN], f32)
            w0_sb = big.tile([128, D_IN // 128, DH], bf16)
            w1_sb = big.tile([DH, DH], f32)
            wf_sb = big.tile([DH, DC], f32)
            b0_sb = big.tile([128, DH], f32)
            b1_sb = big.tile([128, DH], f32)
            bf_sb = big.tile([128, DC], f32)
            iota_sb = big.tile([128, 128], bf16)
            ident = big.tile([128, 128], f32)
            ones_sb = big.tile([128, 1], f32)
            ones_row = big.tile([1, 128], f32)
            stats_sb = big.tile([1, 64], f32)

            hloc = big.tile([128, TN, DH], f32)     # dinv * h (self term)
            agg = big.tile([128, TN, DH], f32)
            state = big.tile([128, TN, DH], f32)
            shard_sb = big.tile([128, TN, DH], bf16)
            final_sb = big.tile([128, TN, DC], f32)
            sq = final_sb[:, :, 0:DH]               # scratch; real use layer 2

            nc.sync.dma_start(out=segrel_sb[:], in_=segrel_d[:])
            nc.sync.dma_start(out=dinv_sb[:], in_=dinv_d[:])
            nc.sync.dma_start(out=w0_sb[:], in_=w0_d.rearrange("(c p) f -> p c f", p=128))
            nc.sync.dma_start(out=w1_sb[:], in_=w1_d[:])
            nc.sync.dma_start(out=wf_sb[:], in_=wf_d[:])
            nc.sync.dma_start(out=b0_sb[:], in_=b0_d[:])
            nc.sync.dma_start(out=b1_sb[:], in_=b1_d[:])
            nc.sync.dma_start(out=bf_sb[:], in_=bf_d[:])
            nc.sync.dma_start(out=iota_sb[:], in_=iota_d[:])
            make_identity(nc, ident[:])
            nc.vector.memset(ones_sb[:], 1.0)
            nc.vector.memset(ones_row[:], 1.0)

            def dinv_b(ap_shape):
                return dinv_sb[:, :, None].to_broadcast(ap_shape)

            # ---------------- phase A: t0 = x @ W0 (bf16), hloc = dinv*t0
            NB = 4
            TH0 = (TN + NB - 1) // NB
            bands = [(b * TH0, min(TN, (b + 1) * TH0)) for b in range(NB)]
            with tc.tile_pool(name="xTp", bufs=1) as xTp:
                for h, (tlo, thi) in enumerate(bands):
                    nh = thi - tlo
                    if nh <= 0:
                        continue
                    xT = [xTp.tile([128, TH0 * 128], bf16, tag=f"xT{c}",
                                   name=f"xT{c}_{h}") for c in range(KC)]
                    for c in range(KC):
                        nc.sync.dma_start_transpose(
                            out=xT[c][:, 0:nh * 128],
                            in_=x_bf[tlo * 128: thi * 128,
                                     128 * c:128 * (c + 1)])
                    for t in range(tlo, thi):
                        t0 = psA.tile([128, DH], f32, tag="psA", name=f"t0_{t}")
                        for c in range(KC):
                            nc.tensor.matmul(
                                out=t0[:],
                                lhsT=xT[c][:, 128 * (t - tlo):128 * (t - tlo + 1)],
                                rhs=w0_sb[:, c, :],
                                start=(c == 0), stop=(c == KC - 1))
                        nc.vector.tensor_tensor(
                            out=hloc[:, t, :], in0=t0[:],
                            in1=dinv_sb[:, t:t + 1].to_broadcast([128, DH]),
                            op=mybir.AluOpType.mult)

            def write_table_allgather_expand():
                nc.vector.tensor_copy(out=shard_sb[:], in_=hloc[:])
                nc.sync.dma_start(
                    out=shard.rearrange("(t p) f -> p t f", p=128),
                    in_=shard_sb[:])
                nc.gpsimd.collective_compute(
                    "AllGather", mybir.AluOpType.bypass,
                    replica_groups=rg,
                    ins=[shard[0:NL, :]],
                    outs=[table[0:N, :]],
                )
                for c in range(NCH):
                    c0 = c * CH
                    nc.sync.dma_start(
                        out=table_p[c0:c0 + chsz[c], 0:DH],
                        in_=table[c0:c0 + chsz[c], :])

            write_table_allgather_expand()

            # ---------------- layers
            for layer in range(3):
                # ---- gather + segment-sum into agg
                for g in range(NG):
                    gt = gp.tile([128, SIC], i16, tag="gidx")
                    nc.sync.dma_start(
                        out=gt[:],
                        in_=gidx_d[:, g * SIC:(g + 1) * SIC])
                    aggsl = agg[:, g * G:(g + 1) * G, :]
                    for c in range(NCH):
                        mc = MCS[c]
                        msg = msgp.tile([128, mc, 128], bf16, tag=f"msg{mc}")
                        nc.gpsimd.dma_gather(
                            out_ap=msg[:],
                            in_ap=table_p[c * CH:c * CH + chsz[c], :],
                            idxs_ap=gt[:, ioff[c]:ioff[c + 1]],
                            num_idxs=NIS[c],
                            num_idxs_reg=NIS[c],
                            elem_size=128,
                            single_packet=False,
                            queue_num=c % 4,
                        )
                        oh = ohp.tile([128, mc, 128], bf16, tag=f"oh{mc}")
                        sc0 = g * SMC + moff[c]
                        nc.vector.tensor_tensor(
                            out=oh[:],
                            in0=iota_sb[:, None, :].to_broadcast([128, mc, 128]),
                            in1=segrel_sb[:, sc0:sc0 + mc, None]
                                .to_broadcast([128, mc, 128]),
                            op=mybir.AluOpType.is_equal)
                        pws = psw.tile([128, G, DH], f32, tag="pw",
                                       name=f"pw_{layer}_{g}_{c}")
                        for wi in range(G):
                            for k in range(KWS[c]):
                                s = wi * KWS[c] + k
                                nc.tensor.matmul(
                                    out=pws[:, wi, :],
                                    lhsT=oh[:, s, :],
                                    rhs=msg[:, s, 0:DH],
                                    start=(k == 0),
                                    stop=(k == KWS[c] - 1))
                        if c == 0:
                            nc.vector.tensor_copy(out=aggsl, in_=pws[:])
                        else:
                            nc.vector.tensor_tensor(
                                out=aggsl, in0=aggsl, in1=pws[:],
                                op=mybir.AluOpType.add)

                # ---- pre = dinv * (agg + hloc)   (reuse agg buffer)
                nc.vector.tensor_tensor(out=agg[:], in0=agg[:], in1=hloc[:],
                                        op=mybir.AluOpType.add)
                nc.vector.tensor_tensor(out=agg[:], in0=agg[:],
                                        in1=dinv_b([128, TN, DH]),
                                        op=mybir.AluOpType.mult)

                # ---- out_k = pre @ W + b  (layer 0: W already applied)
                if layer == 0:
                    nc.vector.tensor_tensor(
                        out=state[:], in0=agg[:],
                        in1=b0_sb[:, None, :].to_broadcast([128, TN, DH]),
                        op=mybir.AluOpType.add)
                else:
                    W_sb, b_sb, DO = ((w1_sb, b1_sb, DH) if layer == 1
                                      else (wf_sb, bf_sb, DC))
                    dst_buf = state if layer == 1 else final_sb
                    for t in range(TN):
                        ptp = pstr.tile([DH, 128], f32, tag="small", name=f"tr{t}")
                        nc.tensor.transpose(out=ptp[:], in_=agg[:, t, :],
                                            identity=ident[:])
                        preT = smp.tile([DH, 128], f32, tag="preT")
                        nc.scalar.activation(out=preT[:], in_=ptp[:],
                                             func=mybir.ActivationFunctionType.Copy)
                        po = pstr.tile([128, DO], f32, tag="small", name=f"po{t}")
                        nc.tensor.matmul(out=po[:], lhsT=preT[:], rhs=W_sb[:, 0:DO],
                                         start=True, stop=True)
                        nc.vector.tensor_tensor(
                            out=dst_buf[:, t, 0:DO], in0=po[:],
                            in1=b_sb[:, 0:DO],
                            op=mybir.AluOpType.add)

                if layer == 2:
                    nc.sync.dma_start(
                        out=out_d[0:(TN - 1) * 128, :].rearrange(
                            "(t p) f -> p t f", p=128),
                        in_=final_sb[:, 0:TN - 1, :])
                    lastn = NL - (TN - 1) * 128
                    nc.sync.dma_start(
                        out=out_d[(TN - 1) * 128: NL, :],
                        in_=final_sb[0:lastn, TN - 1, :])
                    continue

                # ---- pairnorm stats: colsums of state and state^2
                nc.vector.tensor_tensor(out=sq, in0=state[:], in1=state[:],
                                        op=mybir.AluOpType.mult)
                ps_s = psst.tile([DH, 1], f32, tag="st_s")
                ps_q = psst.tile([DH, 1], f32, tag="st_q")
                for t in range(TN):
                    kk = 128 if t < TN - 1 else (NL - (TN - 1) * 128)
                    nc.tensor.matmul(out=ps_s[:], lhsT=state[0:kk, t, :],
                                     rhs=ones_sb[0:kk, :],
                                     start=(t == 0), stop=(t == TN - 1))
                    nc.tensor.matmul(out=ps_q[:], lhsT=sq[0:kk, t, :],
                                     rhs=ones_sb[0:kk, :],
                                     start=(t == 0), stop=(t == TN - 1))
                s_sb = smp.tile([DH, 2], f32, tag="s_sb")
                nc.vector.tensor_copy(out=s_sb[:, 0:1], in_=ps_s[:])
                nc.vector.tensor_copy(out=s_sb[:, 1:2], in_=ps_q[:])
                nc.sync.dma_start(out=st_in.rearrange("o (p f) -> (o p) f", f=2),
                                  in_=s_sb[:])
                nc.gpsimd.collective_compute(
                    "AllReduce", mybir.AluOpType.add, replica_groups=rg,
                    ins=[st_in[:]], outs=[st_out[:]])
                nc.sync.dma_start(out=stats_sb[:], in_=st_out[:])
                # stats_sb[0, 2i] = colsum_i, stats_sb[0, 2i+1] = sqsum_i
                s_ap = stats_sb[:].rearrange("o (p f) -> o p f", f=2)  # [1,32,2]
                mean = smp.tile([1, DH], f32, tag="mean")
                nc.vector.tensor_scalar(
                    out=mean[:], in0=s_ap[:, :, 0], scalar1=1.0 / cfg.N,
                    scalar2=None, op0=mybir.AluOpType.mult)
                m2 = smp.tile([1, DH], f32, tag="m2")
                nc.vector.tensor_tensor(out=m2[:], in0=mean[:],
                                        in1=s_ap[:, :, 0],
                                        op=mybir.AluOpType.mult)  # s_f^2/N
                r1 = smp.tile([1, 1], f32, tag="r1")
                nc.vector.reduce_sum(out=r1[:], in_=m2[:],
                                     axis=mybir.AxisListType.X)
                qs = smp.tile([1, 1], f32, tag="qs")
                nc.vector.reduce_sum(out=qs[:], in_=s_ap[:, :, 1],
                                     axis=mybir.AxisListType.X)
                v_ = smp.tile([1, 1], f32, tag="v_")
                nc.vector.tensor_tensor(out=v_[:], in0=qs[:], in1=r1[:],
                                        op=mybir.AluOpType.subtract)
                nc.vector.tensor_scalar(
                    out=v_[:], in0=v_[:], scalar1=1.0 / cfg.N,
                    scalar2=cfg.EPS, op0=mybir.AluOpType.mult,
                    op1=mybir.AluOpType.add)
                rt = smp.tile([1, 1], f32, tag="rt")
                nc.scalar.activation(out=rt[:], in_=v_[:],
                                     func=mybir.ActivationFunctionType.Sqrt)
                scl = smp.tile([1, 1], f32, tag="scl")
                nc.vector.reciprocal(out=scl[:], in_=rt[:])

                # materialize mean|scale across partitions: ones-matmul bcast
                msc = smp.tile([1, DH + 1], f32, tag="msc")
                nc.vector.tensor_copy(out=msc[:, 0:DH], in_=mean[:])
                nc.vector.tensor_copy(out=msc[:, DH:DH + 1], in_=scl[:])
                pmsc = pstr.tile([128, DH + 1], f32, tag="small", name=f"pmsc{layer}")
                nc.tensor.matmul(out=pmsc[:], lhsT=ones_row[0:1, :],
                                 rhs=msc[:], start=True, stop=True)
                msc128 = smp.tile([128, DH + 1], f32, tag="msc128")
                nc.vector.tensor_copy(out=msc128[:], in_=pmsc[:])

                # ---- z = relu((state - mean) * scale); hloc = dinv * z
                nc.vector.tensor_tensor(
                    out=state[:], in0=state[:],
                    in1=msc128[:, None, 0:DH].to_broadcast([128, TN, DH]),
                    op=mybir.AluOpType.subtract)
                nc.vector.tensor_tensor(
                    out=state[:], in0=state[:],
                    in1=msc128[:, None, DH:DH + 1].to_broadcast([128, TN, DH]),
                    op=mybir.AluOpType.mult)
                nc.vector.tensor_scalar(
                    out=state[:], in0=state[:], scalar1=0.0, scalar2=None,
                    op0=mybir.AluOpType.max)
                nc.vector.tensor_tensor(out=hloc[:], in0=state[:],
                                        in1=dinv_b([128, TN, DH]),
                                        op=mybir.AluOpType.mult)
                write_table_allgather_expand()

    nc.compile()
    return nc, names


# ----------------------------------------------------------------- entry

def make_inputs(cfg, pp, x, W0, b0, W1, b1, Wf, bf):
    """Per-core input maps for run_bass_kernel_spmd."""
    C, NL, NLP = cfg.C, cfg.NL, cfg.NLP
    x_pad = np.zeros((C, NLP, cfg.D_IN), dtype=BF16)
    for c in range(C):
        x_pad[c, :NL] = x[c * NL:(c + 1) * NL].astype(BF16)
    iota_row = np.tile(np.arange(128, dtype=np.float32).astype(BF16)[None, :], (128, 1))
    in_maps = []
    for c in range(C):
        in_maps.append({
            "x_bf": x_pad[c],
            "gidx": pp["gidx"][c],
            "segrel": pp["segrel"][c],
            "dinv_nm": pp["dinv_nm"][c],
            "w0": W0.astype(BF16),
            "w1": W1.astype(np.float32),
            "wf": Wf.astype(np.float32),
            "b0": np.tile(b0.reshape(1, -1).astype(np.float32), (128, 1)),
            "b1": np.tile(b1.reshape(1, -1).astype(np.float32), (128, 1)),
            "bf_": np.tile(bf.reshape(1, -1).astype(np.float32), (128, 1)),
            "iota_row": iota_row,
        })
    return in_maps


_CACHE = {}

def kernel(x, edge_index, W0, b0, W1, b1, Wf, bf):
    from concourse import bass_utils
    cfg = FULL
    x = np.asarray(x)
    edge_index = np.asarray(edge_index)
    pp = preprocess(cfg, edge_index)
    key = ("prog", pp["RS"])
    if key not in _CACHE:
        _CACHE[key] = build_program(cfg, pp["RS"])
    nc, _names = _CACHE[key]
    in_maps = make_inputs(cfg, pp, x, W0, b0, W1, b1, Wf, bf)
    res = bass_utils.run_bass_kernel_spmd(nc, in_maps, list(range(cfg.C)))
    out = np.concatenate([res.results[c]["out"] for c in range(cfg.C)], axis=0)
    return out.astype(np.float32)
